# revision 19
# baseline (speedup 1.0000x reference)
"""Trainium2 Bass kernel for nn_DeeperAttentionGCNSW (GAT x2 + GRU + head).

Strategy (per sharding hint): nodes sharded into 8 contiguous blocks
(2500/core, padded 2560). Edges routed by destination core and sorted by
destination so segment-softmax / scatter-add stay core-local. Per
timestep and per GAT layer:
  M-phase : h|als|ald = x @ W_ext on the PE (attention projections are
            folded into the weight matrix host-side).
  AllGather: each core's table shard (h|als rows, bf16) is gathered so
            every core holds the full 20480-row source table in HBM.
  E-phase : edges (dst-sorted, 128-dst windows, 128-edge chunks) are
            processed with dma_gather (source rows by src id), a
            selection matrix S[e,d]=(dst_local==d) built by iota-compare,
            attention weights w=exp(leakyrelu(als_src+ald_dst)) on the
            ACT engine, and aggregation num|den = S^T @ [h*w | w]
            accumulated in PSUM per window.
GRU runs node-parallel after the 16 steps; final linear produces y.
Host does only edge routing/padding and weight prep; all FLOPs run on
the NeuronCores.
"""
import math

import ml_dtypes
import numpy as np

# ---------------------------------------------------------------- config
NC_CORES = 8
N = 20000
T = 16
E = 320000
F_IN = 128
H = 4          # heads (both layers)
C = 64         # per-head channels (both layers)
HC = H * C     # 256
NEG_SLOPE = 0.2

SH = N // NC_CORES        # 2500 real nodes per core
PC = 2560                 # padded per-core nodes (20 x 128)
NW = PC // 128            # 20 windows (=row tiles) per core
K = 19                    # chunk slots per window (128 edges each)
ROW = 384                 # table row, bf16 elems: h(256) | als(4) | pad
GCOLS = (NW + 2) * K * 8  # gather idx cols (int16 wrapped layout)
DCOLS = (NW + 2) * K      # dst_local cols

_CACHE = {}


# ------------------------------------------------------------ host: prep
def _route_edges(ei_t, cfg):
    """Route/sort/pad one timestep's edges for every core.

    Returns (gidx[NCORES,128,GCOLS] int16, dl[NCORES,128,DCOLS] f32)
    or None on (improbable) window overflow.
    """
    ncores, sh, pc, nw, k = (cfg["ncores"], cfg["sh"], cfg["pc"],
                             cfg["nw"], cfg["k"])
    n = ncores * sh
    src = np.concatenate([ei_t[0], np.arange(n, dtype=np.int64)])
    dst = np.concatenate([ei_t[1], np.arange(n, dtype=np.int64)])
    # table row of a global node id (shards are padded to pc rows)
    trow = (src // sh) * pc + (src % sh)
    core = dst // sh
    dloc = dst - core * sh

    gcols = (nw + 2) * k * 8
    dcols = (nw + 2) * k
    gidx = np.zeros((ncores, 128, gcols), np.int16)
    dl = np.full((ncores, 128, dcols), -1.0, np.float32)

    nslot = (nw + 2) * k * 128
    for c in range(ncores):
        m = core == c
        dl_c = dloc[m]
        tr_c = trow[m]
        order = np.argsort(dl_c, kind="stable")
        dl_c = dl_c[order]
        tr_c = tr_c[order]
        win = dl_c >> 7                       # dst window (128 dsts)
        cnt = np.bincount(win, minlength=nw)
        if cnt.max() > k * 128:
            return None
        # slot position: window base + rank within window
        start = np.zeros(nw, np.int64)
        start[1:] = np.cumsum(cnt)[:-1]
        rank = np.arange(dl_c.size) - start[win]
        slot = win * (k * 128) + rank
        idx_arr = np.zeros(nslot, np.int64)
        dl_arr = np.full(nslot, -1.0, np.float32)
        idx_arr[slot] = tr_c
        dl_arr[slot] = dl_c & 127             # window-relative dst
        # wrapped int16 idx layout: idx i -> [i%16, i//16], replicated
        w16 = idx_arr.reshape(-1, 16).T.astype(np.int16)   # [16, nslot/16]
        gidx[c] = np.tile(w16, (8, 1))
        dl[c] = dl_arr.reshape(-1, 128).T                  # [128, slots/128]
    return gidx, dl


def _prep_host(inputs, cfg):
    """All host-side preprocessing -> per-core in_maps."""
    ncores, sh, pc, t_steps = cfg["ncores"], cfg["sh"], cfg["pc"], cfg["T"]
    fin, h, ch = cfg["fin"], cfg["h"], cfg["c"]
    hc = h * ch
    x = np.asarray(inputs["x_sequence"], np.float32)      # [n, T, fin]
    eis = np.asarray(inputs["edge_index_sequence"]).astype(np.int64)

    def f32(k):
        return np.asarray(inputs[k], np.float32)

    W1, W2 = f32("W1"), f32("W2")
    as1, ad1 = f32("att_src1"), f32("att_dst1")
    as2, ad2 = f32("att_src2"), f32("att_dst2")
    b1, b2 = f32("b1"), f32("b2")
    Wih, Whh = f32("W_ih"), f32("W_hh")
    bih, bhh = f32("b_ih"), f32("b_hh")
    Wout, bout = f32("W_out"), f32("b_out")

    # fold attention projections into the weight matrices
    def ext(W, a_s, a_d):
        Wr = W.reshape(W.shape[0], h, ch)
        wals = np.einsum("fhc,hc->fh", Wr, a_s)
        wald = np.einsum("fhc,hc->fh", Wr, a_d)
        return np.concatenate([W, wals, wald], axis=1)     # [f, hc+2h]

    W1e = ext(W1, as1, ad1)                                # [fin, 264]
    W2e_full = ext(W2, as2, ad2)                           # [hc, 264]
    W2e = W2e_full.reshape(2, hc // 2, hc + 2 * h)

    b1c = np.broadcast_to(b1, (128, hc)).copy()
    b2c = np.broadcast_to(b2, (128, ch)).copy()
    Wiha = np.concatenate([Wih, bih[None, :]], 0)          # [65, 192]
    Whha = np.concatenate([Whh, bhh[None, :]], 0)
    Woutb = np.concatenate([Wout, bout[None, :]], 0)       # [65, 1]

    # per-core xT  [T, 128, pc]
    xts = []
    for c in range(ncores):
        xs = x[c * sh:(c + 1) * sh]                        # [sh, T, fin]
        xt = np.zeros((t_steps, fin, pc), np.float32)
        xt[:, :, :sh] = np.transpose(xs, (1, 2, 0))
        xts.append(xt)

    gidx_all = np.zeros((ncores, t_steps, 128, cfg["gcols"]), np.int16)
    dl_all = np.zeros((ncores, t_steps, 128, cfg["dcols"]), np.float32)
    dlr_all = np.zeros((ncores, t_steps, cfg["dcols"], 128),
                       ml_dtypes.bfloat16)
    for tt in range(t_steps):
        r = _route_edges(eis[tt], cfg)
        if r is None:
            return None
        gidx_all[:, tt], dl_all[:, tt] = r[0], r[1]
        dlr_all[:, tt] = np.transpose(r[1], (0, 2, 1)).astype(ml_dtypes.bfloat16)

    in_maps = []
    for c in range(ncores):
        in_maps.append({
            "xT": xts[c], "gidx": gidx_all[c], "dl": dl_all[c],
            "dlr": dlr_all[c],
            "W1e": W1e, "W2e": W2e, "b1c": b1c, "b2c": b2c,
            "Wiha": Wiha, "Whha": Whha, "Woutb": Woutb,
        })
    return in_maps


# ------------------------------------------------------------ bass kernel
def build_kernel(cfg):
    import concourse.bacc as bacc
    import concourse.mybir as mybir
    import concourse.tile as tile
    from concourse import bass
    from concourse.masks import make_identity

    ncores, pc, nw, k, t_steps = (cfg["ncores"], cfg["pc"], cfg["nw"],
                                  cfg["k"], cfg["T"])
    fin, h, ch = cfg["fin"], cfg["h"], cfg["c"]
    hc = h * ch
    row = cfg["row"]
    gcols, dcols = cfg["gcols"], cfg["dcols"]
    ext_w = hc + 2 * h                                     # 264
    nrows = ncores * pc
    f32, bf16, i16, i32 = (mybir.dt.float32, mybir.dt.bfloat16,
                           mybir.dt.int16, mybir.dt.int32)

    nc = bacc.Bacc("TRN2", target_bir_lowering=False, debug=False,
                   num_devices=ncores)

    # ---- I/O
    xT_d = nc.dram_tensor("xT", [t_steps, fin, pc], f32, kind="ExternalInput")
    gidx_d = nc.dram_tensor("gidx", [t_steps, 128, gcols], i16,
                            kind="ExternalInput")
    dl_d = nc.dram_tensor("dl", [t_steps, 128, dcols], f32,
                          kind="ExternalInput")
    dlr_d = nc.dram_tensor("dlr", [t_steps, dcols, 128], bf16,
                           kind="ExternalInput")
    W1e_d = nc.dram_tensor("W1e", [fin, ext_w], f32, kind="ExternalInput")
    W2e_d = nc.dram_tensor("W2e", [2, hc // 2, ext_w], f32,
                           kind="ExternalInput")
    b1c_d = nc.dram_tensor("b1c", [128, hc], f32, kind="ExternalInput")
    b2c_d = nc.dram_tensor("b2c", [128, ch], f32, kind="ExternalInput")
    Wiha_d = nc.dram_tensor("Wiha", [ch + 1, 3 * ch], f32,
                            kind="ExternalInput")
    Whha_d = nc.dram_tensor("Whha", [ch + 1, 3 * ch], f32,
                            kind="ExternalInput")
    Woutb_d = nc.dram_tensor("Woutb", [ch + 1, 1], f32, kind="ExternalInput")
    y_d = nc.dram_tensor("y", [pc, 1], f32, kind="ExternalOutput")

    # ---- scratch DRAM
    tb_shard = nc.dram_tensor("tb_shard", [pc, row], bf16, kind="Internal")
    tbl_space = "Shared" if ncores > 4 else "Local"
    table = [nc.dram_tensor(f"table{i}", [nrows, row], bf16,
                            kind="Internal", addr_space=tbl_space)
             for i in range(2)]
    gat2_seq = nc.dram_tensor("gat2_seq", [t_steps, 128, nw, ch], f32,
                              kind="Internal")

    rg = [[i for i in range(ncores)]]

    with tile.TileContext(nc) as tc:
        with tc.tile_pool(name="cst", bufs=1) as cst, \
             tc.tile_pool(name="big", bufs=1) as big, \
             tc.tile_pool(name="wrk", bufs=3) as wrk, \
             tc.tile_pool(name="ps", bufs=1, space="PSUM") as ps:

            # ---------- constants
            iota_i = cst.tile([128, 128], i32, tag="iota_i")
            nc.gpsimd.iota(iota_i[:], pattern=[[1, 128]], base=0,
                           channel_multiplier=0)
            iota_f = cst.tile([128, 128], f32, tag="iota_f")
            nc.vector.tensor_copy(iota_f[:], iota_i[:])
            iota2_i = cst.tile([128, 2, 128], i32, tag="iota2_i")
            nc.gpsimd.iota(iota2_i[:], pattern=[[0, 2], [1, 128]], base=0,
                           channel_multiplier=0)
            iota2_f = cst.tile([128, 2, 128], f32, tag="iota2_f")
            nc.vector.tensor_copy(iota2_f[:], iota2_i[:])
            iotap_i2 = cst.tile([128, 256], i32, tag="iotap_i2")
            nc.gpsimd.iota(iotap_i2[:], pattern=[[0, 256]], base=0,
                           channel_multiplier=1)
            iota_pb = cst.tile([128, 2, 128], bf16, tag="iota_pb")
            nc.vector.tensor_copy(
                iota_pb[:], iotap_i2[:].rearrange("p (a b) -> p a b", a=2))
            iotap_i = cst.tile([128, 128], i32, tag="iotap_i")
            nc.gpsimd.iota(iotap_i[:], pattern=[[0, 128]], base=0,
                           channel_multiplier=1)
            iota_p = cst.tile([128, 128], bf16, tag="iota_p")
            nc.vector.tensor_copy(iota_p[:], iotap_i[:])
            ident_b = cst.tile([128, 128], bf16, tag="ident_b")
            make_identity(nc, ident_b[:])
            ident_f = cst.tile([128, 128], f32, tag="ident_f")
            make_identity(nc, ident_f[:])

            W1e_s = cst.tile([fin, ext_w], f32, tag="w1e")
            nc.sync.dma_start(W1e_s[:], W1e_d.ap())
            W2e_s = cst.tile([hc // 2, 2, ext_w], f32, tag="w2e")
            nc.sync.dma_start(
                W2e_s[:], W2e_d.ap().rearrange("b f j -> f b j"))
            b1c_s = cst.tile([128, hc], f32, tag="b1c")
            nc.sync.dma_start(b1c_s[:], b1c_d.ap())
            b2c_s = cst.tile([128, ch], f32, tag="b2c")
            nc.sync.dma_start(b2c_s[:], b2c_d.ap())
            Wiha_s = cst.tile([ch + 1, 3 * ch], f32, tag="wiha")
            nc.sync.dma_start(Wiha_s[:], Wiha_d.ap())
            Whha_s = cst.tile([ch + 1, 3 * ch], f32, tag="whha")
            nc.sync.dma_start(Whha_s[:], Whha_d.ap())
            Woutb_s = cst.tile([ch + 1, 1], f32, tag="woutb")
            nc.sync.dma_start(Woutb_s[:], Woutb_d.ap())

            # ---------- persistent state
            ald1 = big.tile([128, nw, h], bf16, tag="ald1")
            ald2 = big.tile([128, nw, h], bf16, tag="ald2")
            gat1 = big.tile([128, nw, hc], f32, tag="gat1")
            gat2 = big.tile([128, nw, ch], f32, tag="gat2")
            ring = [big.tile([128, k, row], bf16, tag=f"ring{i}",
                             name=f"ring{i}") for i in range(2)]
            dl_sb = big.tile([128, dcols], f32, tag="dl_sb")
            gidx_sb = big.tile([128, gcols], i16, tag="gidx_sb")
            hst = big.tile([128, nw, ch], f32, tag="hst")       # GRU state

            # ---------------- helper: M phase (x/gat1 -> table rows + ald)
            def m_phase(tt, layer):
                ald = ald1 if layer == 1 else ald2
                if layer == 1:
                    x_cur = wrk.tile([128, pc], f32, tag="x_cur")
                    nc.sync.dma_start(x_cur[:], xT_d.ap()[tt])
                for r in range(nw):
                    pm = ps.tile([128, ext_w], f32, tag="pmm", bufs=2)
                    if layer == 1:
                        nc.tensor.matmul(
                            out=pm[:], lhsT=x_cur[:, r * 128:(r + 1) * 128],
                            rhs=W1e_s[:], start=True, stop=True)
                    else:
                        # transpose gat1 row-tile (2 f-blocks of 128)
                        g1t = wrk.tile([128, 2, 128], f32, tag="g1t")
                        for b in range(2):
                            pt = ps.tile([128, 128], f32, tag="ptt", bufs=4)
                            nc.tensor.transpose(
                                out=pt[:],
                                in_=gat1[:, r, b * 128:(b + 1) * 128],
                                identity=ident_f[:])
                            nc.vector.tensor_copy(g1t[:, b, :], pt[:])
                        for b in range(2):
                            nc.tensor.matmul(
                                out=pm[:], lhsT=g1t[:, b, :],
                                rhs=W2e_s[:, b, :],
                                start=(b == 0), stop=(b == 1))
                    tb = wrk.tile([128, hc + h], bf16, tag="tb")
                    nc.vector.tensor_copy(tb[:], pm[:, :hc + h])
                    nc.scalar.copy(ald[:, r, :], pm[:, hc + h:hc + 2 * h])
                    nc.sync.dma_start(
                        tb_shard.ap()[r * 128:(r + 1) * 128, :hc + h], tb[:])

            # ---------------- helper: one pair of edge chunks
            def chunk2(c0, npair, ringbuf, dlbase, dlt, aldw, agg,
                       first, last):
                # S2[e, ci, d] = (dst_local[e, ci] == d)
                S2 = wrk.tile([128, 2, 128], bf16, tag="S2")
                nc.vector.tensor_tensor(
                    out=S2[:, :npair, :],
                    in0=dl_sb[:, dlbase].to_broadcast([128, npair, 128]),
                    in1=iota2_f[:, :npair, :], op=mybir.AluOpType.is_equal)
                # ST2[d, ci, e] = (d == dst_local[e, ci])
                ST2 = wrk.tile([128, 2, 128], bf16, tag="ST2")
                nc.vector.tensor_tensor(
                    out=ST2[:, :npair, :], in0=iota_pb[:, :npair, :],
                    in1=dlt[:, c0:c0 + npair, :],
                    op=mybir.AluOpType.is_equal)
                # ald per edge (one matmul per chunk)
                alde2 = ps.tile([128, 2 * h], f32, tag="ptt", bufs=4)
                for ci in range(npair):
                    nc.tensor.matmul(out=alde2[:, ci * h:(ci + 1) * h],
                                     lhsT=ST2[:, ci, :], rhs=aldw,
                                     start=True, stop=True)
                # logits -> w
                lg2 = wrk.tile([128, 2, h], f32, tag="lg2")
                nc.vector.tensor_add(
                    lg2[:, :npair, :],
                    ringbuf[:, c0:c0 + npair, hc:hc + h],
                    alde2[:, :npair * h].rearrange("p (a b) -> p a b",
                                                   b=h))
                lr2 = wrk.tile([128, 2, h], f32, tag="lr2")
                nc.vector.scalar_tensor_tensor(
                    out=lr2[:, :npair, :], in0=lg2[:, :npair, :],
                    scalar=NEG_SLOPE, in1=lg2[:, :npair, :],
                    op0=mybir.AluOpType.mult, op1=mybir.AluOpType.max)
                msg2 = wrk.tile([128, 2, hc + h], bf16, tag="msg2")
                nc.scalar.activation(msg2[:, :npair, hc:hc + h],
                                     lr2[:, :npair, :],
                                     mybir.ActivationFunctionType.Exp)
                nc.vector.tensor_mul(
                    msg2[:, :npair, :hc], ringbuf[:, c0:c0 + npair, :hc],
                    msg2[:, :npair, hc:hc + h].to_broadcast(
                        [128, npair, h, ch]))
                for ci in range(npair):
                    nc.tensor.matmul(
                        out=agg[:], lhsT=S2[:, ci, :],
                        rhs=msg2[:, ci, :],
                        start=(first and ci == 0),
                        stop=(last and ci == npair - 1))

            # ---------------- helper: E phase
            def e_phase(tt, layer, tbl):  # noqa: C901
                ald = ald1 if layer == 1 else ald2

                gsub = 4  # chunks per dma_gather (512 idxs)

                def gather(w, half):
                    for g0 in range(0, k, gsub):
                        gn = min(gsub, k - g0)
                        nc.gpsimd.dma_gather(
                            out_ap=ring[half][:, g0:g0 + gn, :],
                            in_ap=tbl.ap(),
                            idxs_ap=gidx_sb[:, bass.ds(
                                w * (k * 8) + g0 * 8, gn * 8)],
                            num_idxs=gn * 128, num_idxs_reg=gn * 128,
                            elem_size=row)

                gather(0, 0)
                gather(1, 1)

                def win_body(j, w_off, half):
                    # w = 2*j + w_off
                    agg = ps.tile([128, hc + h], f32, tag=f"agg{half}", bufs=1)
                    aldw = ald[:, bass.ds(j * 2 + w_off, 1), :]
                    dlt = wrk.tile([128, k, 128], bf16, tag=f"dlt{half}",
                                   bufs=2)
                    nc.sync.dma_start(
                        dlt[:],
                        dlr_d.ap()[tt][bass.ds((j * 2 + w_off) * k, k), :]
                        .rearrange("a b -> () a b")
                        .to_broadcast([128, k, 128]))
                    npairs = (k + 1) // 2
                    for cp in range(npairs):
                        c0 = cp * 2
                        npair = min(2, k - c0)
                        dlbase = bass.ds((j * 2 + w_off) * k + c0, npair)
                        chunk2(c0, npair, ring[half], dlbase, dlt, aldw,
                               agg, cp == 0, cp == npairs - 1)
                    # prefetch window w+2
                    for g0 in range(0, k, gsub):
                        gn = min(gsub, k - g0)
                        nc.gpsimd.dma_gather(
                            out_ap=ring[half][:, g0:g0 + gn, :],
                            in_ap=tbl.ap(),
                            idxs_ap=gidx_sb[:, bass.ds(
                                (j * 2 + w_off + 2) * (k * 8) + g0 * 8,
                                gn * 8)],
                            num_idxs=gn * 128, num_idxs_reg=gn * 128,
                            elem_size=row)
                    # flush
                    rden = wrk.tile([128, h], f32, tag="rden")
                    nc.vector.reciprocal(rden[:], agg[:, hc:hc + h])
                    if layer == 1:
                        o = wrk.tile([128, hc], f32, tag="o1")
                        nc.vector.tensor_mul(
                            o[:], agg[:, :hc],
                            rden[:].to_broadcast([128, h, ch]))
                        nc.vector.tensor_add(o[:], o[:], b1c_s[:])
                        nc.vector.tensor_scalar_max(
                            gat1[:, bass.ds(j * 2 + w_off, 1), :], o[:], 0.0)
                    else:
                        o = wrk.tile([128, hc], f32, tag="o2")
                        nc.vector.tensor_mul(
                            o[:], agg[:, :hc],
                            rden[:].to_broadcast([128, h, ch]))
                        m1 = wrk.tile([128, ch], f32, tag="m1")
                        nc.vector.tensor_add(m1[:], o[:, 0:ch],
                                             o[:, ch:2 * ch])
                        nc.vector.tensor_add(m1[:], m1[:],
                                             o[:, 2 * ch:3 * ch])
                        nc.vector.tensor_add(m1[:], m1[:],
                                             o[:, 3 * ch:4 * ch])
                        nc.vector.scalar_tensor_tensor(
                            out=m1[:], in0=m1[:], scalar=0.25,
                            in1=b2c_s[:], op0=mybir.AluOpType.mult,
                            op1=mybir.AluOpType.add)
                        nc.vector.tensor_scalar_max(
                            gat2[:, bass.ds(j * 2 + w_off, 1), :],
                            m1[:], 0.0)

                with tc.For_i(0, nw // 2) as j:
                    win_body(j, 0, 0)
                    win_body(j, 1, 1)

            # ================= main program =================
            for tt in range(t_steps):
                nc.sync.dma_start(dl_sb[:], dl_d.ap()[tt])
                nc.sync.dma_start(gidx_sb[:], gidx_d.ap()[tt])
                m_phase(tt, 1)
                nc.gpsimd.collective_compute(
                    "AllGather", mybir.AluOpType.bypass,
                    ins=[tb_shard[:]], outs=[table[(2 * tt) % 2][:]],
                    replica_groups=rg)
                e_phase(tt, 1, table[(2 * tt) % 2])
                m_phase(tt, 2)
                nc.gpsimd.collective_compute(
                    "AllGather", mybir.AluOpType.bypass,
                    ins=[tb_shard[:]], outs=[table[(2 * tt + 1) % 2][:]],
                    replica_groups=rg)
                e_phase(tt, 2, table[(2 * tt + 1) % 2])
                nc.sync.dma_start(gat2_seq.ap()[tt], gat2[:])

            # ================= GRU over time =================
            nc.vector.memset(hst[:], 0.0)
            g2T = [big.tile([ch + 1, 128], f32, tag=f"g2T{i}", name=f"g2T{i}")
                   for i in range(2)]
            hT = [big.tile([ch + 1, 128], f32, tag=f"hT{i}", name=f"hT{i}")
                  for i in range(2)]
            for i in range(2):
                nc.vector.memset(g2T[i][ch:ch + 1, :], 1.0)
                nc.vector.memset(hT[i][ch:ch + 1, :], 1.0)

            g2t_in = big.tile([128, nw, ch], f32, tag="g2t_in")
            for tt in range(t_steps):
                nc.sync.dma_start(g2t_in[:], gat2_seq.ap()[tt])
                for r in range(nw):
                    pr = r % 2
                    ptA = ps.tile([ch, 128], f32, tag="ptt", bufs=4)
                    nc.tensor.transpose(out=ptA[:], in_=g2t_in[:, r, :],
                                        identity=ident_f[:])
                    nc.vector.tensor_copy(g2T[pr][:ch, :], ptA[:])
                    ptB = ps.tile([ch, 128], f32, tag="ptt", bufs=4)
                    nc.tensor.transpose(out=ptB[:], in_=hst[:, r, :],
                                        identity=ident_f[:])
                    nc.vector.tensor_copy(hT[pr][:ch, :], ptB[:])
                    # r|z gates: sigmoid(gi_rz + gh_rz)
                    prz = ps.tile([128, 2 * ch], f32, tag="agg0", bufs=1)
                    nc.tensor.matmul(out=prz[:], lhsT=g2T[pr][:],
                                     rhs=Wiha_s[:, :2 * ch],
                                     start=True, stop=False)
                    nc.tensor.matmul(out=prz[:], lhsT=hT[pr][:],
                                     rhs=Whha_s[:, :2 * ch],
                                     start=False, stop=True)
                    srz = wrk.tile([128, 2 * ch], f32, tag="srz")
                    nc.scalar.activation(
                        srz[:], prz[:], mybir.ActivationFunctionType.Sigmoid)
                    # n gate
                    pgi = ps.tile([128, ch], f32, tag="agg1", bufs=1)
                    nc.tensor.matmul(out=pgi[:], lhsT=g2T[pr][:],
                                     rhs=Wiha_s[:, 2 * ch:],
                                     start=True, stop=True)
                    pgh = ps.tile([128, ch], f32, tag="pmm", bufs=2)
                    nc.tensor.matmul(out=pgh[:], lhsT=hT[pr][:],
                                     rhs=Whha_s[:, 2 * ch:],
                                     start=True, stop=True)
                    gt = wrk.tile([128, ch], f32, tag="gt")
                    nc.vector.tensor_mul(gt[:], pgh[:], srz[:, :ch])
                    nc.vector.tensor_add(gt[:], gt[:], pgi[:])
                    nc.scalar.activation(
                        gt[:], gt[:], mybir.ActivationFunctionType.Tanh)
                    # h = g + z*(h - g)
                    dt_ = wrk.tile([128, ch], f32, tag="dt_")
                    nc.vector.tensor_sub(dt_[:], hst[:, r, :], gt[:])
                    nc.vector.tensor_mul(dt_[:], dt_[:], srz[:, ch:])
                    nc.vector.tensor_add(hst[:, r, :], gt[:], dt_[:])

            # ================= output head =================
            y_sb = big.tile([128, nw], f32, tag="y_sb")
            for r in range(nw):
                pr = r % 2
                ptC = ps.tile([ch, 128], f32, tag="ptt", bufs=4)
                nc.tensor.transpose(out=ptC[:], in_=hst[:, r, :],
                                    identity=ident_f[:])
                nc.vector.tensor_copy(hT[pr][:ch, :], ptC[:])
                py = ps.tile([128, 1], f32, tag="ptt", bufs=4)
                nc.tensor.matmul(out=py[:], lhsT=hT[pr][:], rhs=Woutb_s[:],
                                 start=True, stop=True)
                nc.vector.tensor_copy(y_sb[:, r:r + 1], py[:])
            nc.sync.dma_start(
                y_d.ap().rearrange("(k p) o -> p k o", p=128), y_sb[:])

    nc.compile()
    return nc


def _cfg_full():
    return dict(ncores=NC_CORES, sh=SH, pc=PC, nw=NW, k=K, T=T,
                fin=F_IN, h=H, c=C, row=ROW, gcols=GCOLS, dcols=DCOLS)


# ------------------------------------------------------------ host fallback
def _host_fallback(inputs):
    import scipy.sparse as sp
    x = np.asarray(inputs["x_sequence"], np.float32)
    eis = np.asarray(inputs["edge_index_sequence"]).astype(np.int64)
    f = {k: np.asarray(inputs[k], np.float32) for k in inputs
         if k not in ("x_sequence", "edge_index_sequence")}
    n = x.shape[0]
    loops = np.arange(n, dtype=np.int64)

    def gat(xt, src, dst, W, a_s, a_d, b, heads, out_ch, concat):
        hh = (xt @ W).reshape(n, heads, out_ch)
        als = np.einsum("nhc,hc->nh", hh, a_s)
        ald = np.einsum("nhc,hc->nh", hh, a_d)
        e = als[src] + ald[dst]
        e = np.where(e >= 0, e, NEG_SLOPE * e)
        w = np.exp(e)
        out = np.empty((n, heads, out_ch), np.float32)
        for q in range(heads):
            A = sp.csr_matrix((w[:, q], (dst, src)), shape=(n, n))
            den = np.asarray(A.sum(axis=1)).ravel()
            out[:, q] = (A @ np.ascontiguousarray(hh[:, q])) / (
                den[:, None] + 1e-16)
        out = out.reshape(n, heads * out_ch) if concat else out.mean(1)
        return out + b

    g_out = np.empty((x.shape[1], n, C), np.float32)
    for tt in range(x.shape[1]):
        src = np.concatenate([eis[tt, 0], loops])
        dst = np.concatenate([eis[tt, 1], loops])
        hh = np.maximum(gat(x[:, tt], src, dst, f["W1"], f["att_src1"],
                            f["att_dst1"], f["b1"], H, C, True), 0.0)
        hh = np.maximum(gat(hh, src, dst, f["W2"], f["att_src2"],
                            f["att_dst2"], f["b2"], H, C, False), 0.0)
        g_out[tt] = hh
    hs = np.zeros((n, C), np.float32)
    for tt in range(x.shape[1]):
        gi = g_out[tt] @ f["W_ih"] + f["b_ih"]
        gh = hs @ f["W_hh"] + f["b_hh"]
        r = 1 / (1 + np.exp(-(gi[:, :C] + gh[:, :C])))
        z = 1 / (1 + np.exp(-(gi[:, C:2 * C] + gh[:, C:2 * C])))
        g = np.tanh(gi[:, 2 * C:] + r * gh[:, 2 * C:])
        hs = (1 - z) * g + z * hs
    return hs @ f["W_out"] + f["b_out"]


# ------------------------------------------------------------ entry point
def kernel(**inputs):
    cfg = _cfg_full()
    in_maps = _prep_host(inputs, cfg)
    if in_maps is None:                       # window overflow (improbable)
        return _host_fallback(inputs).squeeze(-1).astype(np.float32)

    if "nc" not in _CACHE:
        _CACHE["nc"] = build_kernel(cfg)
    nc = _CACHE["nc"]

    from concourse.bass_utils import run_bass_kernel_spmd
    res = run_bass_kernel_spmd(nc, in_maps, core_ids=list(range(NC_CORES)))
    out = np.empty(N, np.float32)
    for c in range(NC_CORES):
        out[c * SH:(c + 1) * SH] = res.results[c]["y"][:SH, 0]
    return out
